# revision 41
# baseline (speedup 1.0000x reference)
"""Trainium2 Bass kernel for a 2-layer GRU cell (seq len 1) + actor/critic heads.

Reference computation (PyTorch GRU gate order r,z,n), B=32768, I=128, H=256:
    h0' = GRUCell(x,  h[0]; w_ih0, w_hh0, b_ih0, b_hh0)
    h1' = GRUCell(h0', h[1]; w_ih1, w_hh1, b_ih1, b_hh1)
    logits = h1' @ w_p.T + b_p ; value = h1' @ w_v.T + b_v
Outputs: (logits [B,32], value [B], h_new [2,B,256])

Strategy: pure data parallel over 8 NeuronCores (4096 rows each).  On-chip
compute is feature-major (features on partitions, batch on the free dim):
batch-major DRAM tiles are transposed on the PE, all matmuls run in
float32r (full-rate on TRN2; fp32 matmul is 4 cycles/row), gate math is
fp32 on ACT/DVE/GPSIMD, outputs are PE-transposed back to batch-major.
"""

import functools
import os
import sys

import numpy as np

for _p in ("/opt/trn_rl_repo", "/root/.axon_site/_ro/trn_rl_repo"):
    if os.path.isdir(_p) and _p not in sys.path:
        sys.path.insert(0, _p)

B, S, I, H, A, L = 32768, 1, 128, 256, 32, 2
G = 3 * H  # 768 gate rows
NCORES = 8
BC = B // NCORES  # batch rows per core

# bias pack column indices
_RZ0, _IN0, _HN0 = 0, 4, 6
_RZ1, _IN1, _HN1 = 8, 12, 14
_PV = 16

last_results = None  # BassKernelResults of the most recent run (for profiling)


def _emit(nc, tc, io, bc, nb):
    """Emit the per-core program as a 3-stage software pipeline over tiles:
    S1 load+transpose(t+1) | S2 layer0(t+1) | S3 layer1+heads+stores(t),
    so PE crunches tile t+1 transposes/matmuls while tile t's gate math
    drains through ACT/DVE/GPSIMD."""
    import concourse.bass as bass  # noqa: F401
    from contextlib import ExitStack
    from concourse import mybir

    ctx = ExitStack()

    f32 = mybir.dt.float32
    f32r = mybir.dt.float32r
    bf16 = mybir.dt.bfloat16
    Act = mybir.ActivationFunctionType
    Alu = mybir.AluOpType
    nt = bc // nb
    nblk = nb // 128  # 128-row blocks per batch tile

    def asf(ap):  # read an fp32r tile as plain fp32 (free bit view)
        return ap.bitcast(f32)

    singles = ctx.enter_context(tc.tile_pool(name="singles", bufs=1))
    in_bm = ctx.enter_context(tc.tile_pool(name="in_bm", bufs=3))
    fm = ctx.enter_context(tc.tile_pool(name="fm", bufs=3))
    l1in = ctx.enter_context(tc.tile_pool(name="l1in", bufs=3))
    gt = ctx.enter_context(tc.tile_pool(name="gt", bufs=2))
    out_bm = ctx.enter_context(tc.tile_pool(name="out_bm", bufs=2))
    ps_t = ctx.enter_context(tc.tile_pool(name="ps_t", bufs=3, space="PSUM"))
    ps_g = ctx.enter_context(tc.tile_pool(name="ps_g", bufs=5, space="PSUM"))

    # ---- constants ----
    wih0 = singles.tile([128, G], bf16, tag="wih0")        # w_ih0.T (bf16)
    whh0 = singles.tile([128, 2, G], f32r, tag="whh0")     # w_hh0.T k-chunked
    wih1 = singles.tile([128, 2, G], f32r, tag="wih1")
    whh1 = singles.tile([128, 2, G], f32r, tag="whh1")
    wpv = singles.tile([128, 2, A + 1], f32r, tag="wpv")   # [w_p;w_v].T
    bias = singles.tile([128, 17], f32, tag="bias")
    identr = singles.tile([128, 128], f32r, tag="identr")
    identb = singles.tile([128, 128], bf16, tag="identb")
    for name, t in (("identr", identr), ("identb", identb), ("bias", bias)):
        nc.sync.dma_start(out=t, in_=io[name])
    # weights go down the ACT HWDGE ring so tile-0 input loads (sync ring)
    # are not queued behind 2.8 MB of constants
    for name, t in (("wih0", wih0), ("whh0", whh0), ("wih1", wih1),
                    ("whh1", whh1), ("wpv", wpv)):
        nc.scalar.dma_start(out=t, in_=io[name])

    def transpose(ps_out, sb_in, ident):
        kp = sb_in.partition_size()
        nc.tensor.transpose(ps_out, sb_in, ident[0:kp, 0:kp])

    def load_transpose(t):
        r0, r1 = t * nb, (t + 1) * nb
        x_bm = in_bm.tile([128, nblk, I], bf16, tag="x_bm")
        h0_bm = in_bm.tile([128, nblk, H], f32r, tag="h0_bm")
        h1_bm = in_bm.tile([128, nblk, H], f32r, tag="h1_bm")
        # SWDGE casts f32 -> bf16 during the x load
        nc.gpsimd.dma_start(out=x_bm, in_=io["x"][r0:r1, :].rearrange(
            "(b p) i -> p b i", p=128))
        nc.sync.dma_start(out=h0_bm, in_=io["h0"][r0:r1, :].rearrange(
            "(b p) j -> p b j", p=128))
        nc.sync.dma_start(out=h1_bm, in_=io["h1"][r0:r1, :].rearrange(
            "(b p) j -> p b j", p=128))

        xT = fm.tile([128, nb], bf16, tag="xT")
        h0T = fm.tile([128, 2, nb], f32r, tag="h0T")
        h1T = fm.tile([128, 2, nb], f32r, tag="h1T")
        xps_f = ps_t.tile([128, nb], f32r, tag="tp")
        xps = xps_f.bitcast(bf16)[:, 0:nb]
        for b in range(nblk):
            transpose(xps[:, b * 128:(b + 1) * 128], x_bm[:, b, :], identb)
        nc.scalar.copy(xT, xps)
        for src, dst in ((h0_bm, h0T), (h1_bm, h1T)):
            for c in range(2):
                hps = ps_t.tile([128, nb], f32r, tag="tp")
                for b in range(nblk):
                    transpose(hps[:, b * 128:(b + 1) * 128],
                              src[:, b, c * 128:(c + 1) * 128], identr)
                nc.scalar.copy(dst[:, c, :], hps)
        return {"xT": xT, "h0T": h0T, "h1T": h1T}

    def gru_layer(inp_chunks, hT, wih, whh, rz_col, in_col, hn_col,
                  houtT):
        n_in = len(inp_chunks)
        # r/z gates: 4 chunks of 128 gate rows, PSUM-accumulated
        rz = gt.tile([128, 4, nb], f32, tag="rz")
        for m in range(4):
            psg = ps_g.tile([128, nb], f32, tag="ps")
            for k in range(n_in):
                nc.tensor.matmul(psg, wih[:, k, m * 128:(m + 1) * 128]
                                 if n_in > 1 else
                                 wih[:, m * 128:(m + 1) * 128],
                                 inp_chunks[k], start=(k == 0), stop=False)
            for k in range(2):
                nc.tensor.matmul(psg, whh[:, k, m * 128:(m + 1) * 128],
                                 hT[:, k, :], start=False, stop=(k == 1))
            with tc.high_priority():
                nc.scalar.activation(rz[:, m, :], psg, Act.Sigmoid,
                                     bias=bias[:, rz_col + m:rz_col + m + 1])

        # n gate per 128-feature chunk
        n_sb = gt.tile([128, 2, nb], f32, tag="n_sb")
        d_sb = gt.tile([128, 2, nb], f32, tag="d_sb")
        p_sb = gt.tile([128, 2, nb], f32, tag="p_sb")
        for c in range(2):
            mm = 4 + c
            psi = ps_g.tile([128, nb], f32, tag="ps")
            psh = ps_g.tile([128, nb], f32, tag="ps")
            for k in range(n_in):
                nc.tensor.matmul(psi, wih[:, k, mm * 128:(mm + 1) * 128]
                                 if n_in > 1 else
                                 wih[:, mm * 128:(mm + 1) * 128],
                                 inp_chunks[k], start=(k == 0),
                                 stop=(k == n_in - 1))
            for k in range(2):
                nc.tensor.matmul(psh, whh[:, k, mm * 128:(mm + 1) * 128],
                                 hT[:, k, :], start=(k == 0), stop=(k == 1))
            # chain ops get scheduling priority so later-emitted filler
            # (next tiles' casts/copies) doesn't queue ahead of them
            with tc.high_priority():
                # p = (h_n + b_hhn) * r
                nc.vector.scalar_tensor_tensor(
                    p_sb[:, c, :], psh, bias[:, hn_col + c:hn_col + c + 1],
                    rz[:, c, :], op0=Alu.add, op1=Alu.mult)
                # u = (i_n + b_ihn) + p  (overwrites p)
                nc.vector.scalar_tensor_tensor(
                    p_sb[:, c, :], psi, bias[:, in_col + c:in_col + c + 1],
                    p_sb[:, c, :], op0=Alu.add, op1=Alu.add)
                # n = tanh(u)
                nc.scalar.activation(n_sb[:, c, :], p_sb[:, c, :], Act.Tanh)
                # h' = n + z*(h - n); the two chunks' chains run on
                # different engines (GPSIMD / DVE) concurrently
                eng = nc.gpsimd if c == 0 else nc.vector
                eng.tensor_sub(d_sb[:, c, :], asf(hT[:, c, :]),
                               n_sb[:, c, :])
                eng.tensor_mul(d_sb[:, c, :], rz[:, 2 + c, :],
                               d_sb[:, c, :])
                eng.tensor_add(houtT[:, c, :], n_sb[:, c, :],
                               d_sb[:, c, :])
        return houtT

    def layer0(t, s):
        h0nT = l1in.tile([128, 2, nb], f32r, tag="h0nT")
        gru_layer([s["xT"]], s["h0T"], wih0, whh0,
                  _RZ0, _IN0, _HN0, h0nT)
        return h0nT

    def layer1_gates(t, s, h0nT):
        h1nT = l1in.tile([128, 2, nb], f32r, tag="h1nT")
        gru_layer([h0nT[:, 0, :], h0nT[:, 1, :]], s["h1T"],
                  wih1, whh1, _RZ1, _IN1, _HN1, h1nT)
        return h1nT

    def out_store(t, hsrc, oname, li):
        r0, r1 = t * nb, (t + 1) * nb
        ho_bm = out_bm.tile([128, nblk, H], f32, tag=f"ho{li}")
        for c in range(2):
            ops = ps_t.tile([128, nb], f32r, tag="tp")
            for b in range(nblk):
                transpose(ops[:, b * 128:(b + 1) * 128],
                          hsrc[:, c, b * 128:(b + 1) * 128], identr)
            csrc = asf(ops).rearrange("p (b j) -> p b j", b=nblk)
            cdst = ho_bm[:, :, c * 128:(c + 1) * 128]
            if c == 0:
                nc.scalar.copy(cdst, csrc)
            else:
                nc.vector.tensor_copy(cdst, csrc)
        nc.scalar.dma_start(out=io[oname][r0:r1, :].rearrange(
            "(b p) j -> p b j", p=128), in_=ho_bm)

    def heads_outs(t, h0nT, h1nT):
        r0, r1 = t * nb, (t + 1) * nb
        # h0n stores first: they only need layer-0's chain, so they give
        # PE work while layer-1's chain finishes producing h1nT
        out_store(t, h0nT, "h0n", 0)

        ps3 = ps_g.tile([A + 1, nb], f32, tag="ps")
        for k in range(2):
            nc.tensor.matmul(ps3, wpv[:, k, :], h1nT[:, k, :],
                             start=(k == 0), stop=(k == 1))
        pv = gt.tile([A + 1, nb], f32, tag="pv")
        nc.scalar.activation(pv, ps3, Act.Identity,
                             bias=bias[0:A + 1, _PV:_PV + 1])
        nc.scalar.dma_start(out=io["value"][0:1, r0:r1], in_=pv[A:A + 1, :])
        nc.scalar.dma_start(out=io["logitsT"][:, r0:r1], in_=pv[0:A, :])

        out_store(t, h1nT, "h1n", 1)

    # ---- software pipeline, 2 tiles deep: between tile t's layer-1
    # matmuls and its heads/stores (which wait on the L1 gate chain), PE
    # runs tile t+2's transposes and layer-0 matmuls; tile t+1's layer-0
    # chain likewise drains behind tile t's tail work ----
    s = {0: load_transpose(0)}
    l0 = {0: layer0(0, s[0])}
    if nt > 1:
        s[1] = load_transpose(1)
        l0[1] = layer0(1, s[1])
    h1 = {}
    for t in range(nt):
        h1[t] = layer1_gates(t, s[t], l0[t])
        if t + 2 < nt:
            s[t + 2] = load_transpose(t + 2)
            l0[t + 2] = layer0(t + 2, s[t + 2])
        # heads/stores delayed one tile: by the time they enter the PE
        # stream their h1nT is long done, so PE never waits on the chain
        if t > 0:
            heads_outs(t - 1, l0[t - 1], h1[t - 1])
    heads_outs(nt - 1, l0[nt - 1], h1[nt - 1])

    ctx.close()


@functools.lru_cache(maxsize=2)
def _build(bc=BC, nb=512):
    import concourse.bacc as bacc
    import concourse.tile as tile
    from concourse import mybir

    f32 = mybir.dt.float32
    f32r = mybir.dt.float32r
    bf16 = mybir.dt.bfloat16
    nc = bacc.Bacc("TRN2", target_bir_lowering=False, debug=False)
    io = {}
    for name, shape in (("h0", [bc, H]), ("h1", [bc, H]),
                        ("whh0", [128, 2, G]),
                        ("wih1", [128, 2, G]), ("whh1", [128, 2, G]),
                        ("wpv", [128, 2, A + 1]), ("identr", [128, 128])):
        io[name] = nc.dram_tensor(name, shape, f32r,
                                  kind="ExternalInput").ap()
    io["x"] = nc.dram_tensor("x", [bc, I], f32, kind="ExternalInput").ap()
    io["wih0"] = nc.dram_tensor("wih0", [128, G], bf16,
                                kind="ExternalInput").ap()
    io["identb"] = nc.dram_tensor("identb", [128, 128], bf16,
                                  kind="ExternalInput").ap()
    io["bias"] = nc.dram_tensor("bias", [128, 17], f32,
                                kind="ExternalInput").ap()
    for name, shape in (("logitsT", [A, bc]), ("value", [1, bc]),
                        ("h0n", [bc, H]), ("h1n", [bc, H])):
        io[name] = nc.dram_tensor(name, shape, f32, kind="ExternalOutput").ap()
    with tile.TileContext(nc) as tc:
        _emit(nc, tc, io, bc, nb)
    nc.compile()
    return nc


def _prep_const(w_ih0, w_hh0, b_ih0, b_hh0, w_ih1, w_hh1, b_ih1, b_hh1,
                w_p, b_p, w_v, b_v):
    f = np.float32

    def kchunk(wT):  # [2K,M] -> [128, 2, M]
        return np.ascontiguousarray(
            wT.reshape(2, 128, wT.shape[1]).transpose(1, 0, 2))

    bias = np.zeros((128, 17), f)
    for col, vec in ((_RZ0, (b_ih0 + b_hh0)[:512]), (_RZ1, (b_ih1 + b_hh1)[:512])):
        bias[:, col:col + 4] = vec.reshape(4, 128).T
    for col, vec in ((_IN0, b_ih0[512:]), (_HN0, b_hh0[512:]),
                     (_IN1, b_ih1[512:]), (_HN1, b_hh1[512:])):
        bias[:, col:col + 2] = vec.reshape(2, 128).T
    bias[:A + 1, _PV] = np.concatenate([b_p, b_v])

    import ml_dtypes
    bf = ml_dtypes.bfloat16
    wpv = np.concatenate([w_p, w_v], axis=0).T.astype(f)  # [256, 33]
    return {
        "wih0": np.ascontiguousarray(w_ih0.T.astype(bf)),
        "whh0": kchunk(w_hh0.T.astype(f)),
        "wih1": kchunk(w_ih1.T.astype(f)),
        "whh1": kchunk(w_hh1.T.astype(f)),
        "wpv": kchunk(wpv),
        "bias": bias,
        "identr": np.eye(128, dtype=f),
        "identb": np.eye(128, dtype=bf),
    }


def kernel(x, h, w_ih0, w_hh0, b_ih0, b_hh0, w_ih1, w_hh1, b_ih1, b_hh1,
           w_p, b_p, w_v, b_v):
    global last_results
    from concourse import bass_utils

    args = [np.asarray(a, dtype=np.float32) for a in (
        x, h, w_ih0, w_hh0, b_ih0, b_hh0, w_ih1, w_hh1, b_ih1, b_hh1,
        w_p, b_p, w_v, b_v)]
    x, h = args[0], args[1]
    const = _prep_const(*args[2:])

    nc = _build()
    in_maps = []
    for c in range(NCORES):
        sl = slice(c * BC, (c + 1) * BC)
        m = dict(const)
        m["x"] = np.ascontiguousarray(x[sl, -1, :])
        m["h0"] = np.ascontiguousarray(h[0, sl, :])
        m["h1"] = np.ascontiguousarray(h[1, sl, :])
        in_maps.append(m)

    trace = bool(int(os.environ.get("KERNEL_PROFILE", "0")))
    res = bass_utils.run_bass_kernel_spmd(
        nc, in_maps, core_ids=list(range(NCORES)), trace=trace)
    last_results = res

    logits = np.concatenate(
        [np.ascontiguousarray(r["logitsT"].T) for r in res.results], axis=0)
    value = np.concatenate([r["value"][0] for r in res.results], axis=0)
    h_new = np.stack([
        np.concatenate([r["h0n"] for r in res.results], axis=0),
        np.concatenate([r["h1n"] for r in res.results], axis=0)], axis=0)
    return logits, value, h_new


# revision 42
# speedup vs baseline: 1.2147x; 1.2147x over previous
"""Trainium2 Bass kernel for a 2-layer GRU cell (seq len 1) + actor/critic heads.

Reference computation (PyTorch GRU gate order r,z,n), B=32768, I=128, H=256:
    h0' = GRUCell(x,  h[0]; w_ih0, w_hh0, b_ih0, b_hh0)
    h1' = GRUCell(h0', h[1]; w_ih1, w_hh1, b_ih1, b_hh1)
    logits = h1' @ w_p.T + b_p ; value = h1' @ w_v.T + b_v
Outputs: (logits [B,32], value [B], h_new [2,B,256])

Strategy: pure data parallel over 8 NeuronCores (4096 rows each).  On-chip
compute is feature-major (features on partitions, batch on the free dim):
batch-major DRAM tiles are transposed on the PE, all matmuls run in
float32r (full-rate on TRN2; fp32 matmul is 4 cycles/row), gate math is
fp32 on ACT/DVE/GPSIMD, outputs are PE-transposed back to batch-major.
"""

import functools
import os
import sys

import numpy as np

for _p in ("/opt/trn_rl_repo", "/root/.axon_site/_ro/trn_rl_repo"):
    if os.path.isdir(_p) and _p not in sys.path:
        sys.path.insert(0, _p)

B, S, I, H, A, L = 32768, 1, 128, 256, 32, 2
G = 3 * H  # 768 gate rows
NCORES = 8
BC = B // NCORES  # batch rows per core

# bias pack column indices
_RZ0, _IN0, _HN0 = 0, 4, 6
_RZ1, _IN1, _HN1 = 8, 12, 14
_PV = 16

last_results = None  # BassKernelResults of the most recent run (for profiling)


def _emit(nc, tc, io, bc, nb):
    """Emit the per-core program as a 3-stage software pipeline over tiles:
    S1 load+transpose(t+1) | S2 layer0(t+1) | S3 layer1+heads+stores(t),
    so PE crunches tile t+1 transposes/matmuls while tile t's gate math
    drains through ACT/DVE/GPSIMD."""
    import concourse.bass as bass  # noqa: F401
    from contextlib import ExitStack
    from concourse import mybir

    ctx = ExitStack()

    f32 = mybir.dt.float32
    f32r = mybir.dt.float32r
    bf16 = mybir.dt.bfloat16
    Act = mybir.ActivationFunctionType
    Alu = mybir.AluOpType
    nt = bc // nb
    nblk = nb // 128  # 128-row blocks per batch tile

    def asf(ap):  # read an fp32r tile as plain fp32 (free bit view)
        return ap.bitcast(f32)

    singles = ctx.enter_context(tc.tile_pool(name="singles", bufs=1))
    in_bm = ctx.enter_context(tc.tile_pool(name="in_bm", bufs=3))
    fm = ctx.enter_context(tc.tile_pool(name="fm", bufs=3))
    l1in = ctx.enter_context(tc.tile_pool(name="l1in", bufs=3))
    gt = ctx.enter_context(tc.tile_pool(name="gt", bufs=2))
    out_bm = ctx.enter_context(tc.tile_pool(name="out_bm", bufs=2))
    ps_t = ctx.enter_context(tc.tile_pool(name="ps_t", bufs=3, space="PSUM"))
    ps_g = ctx.enter_context(tc.tile_pool(name="ps_g", bufs=5, space="PSUM"))

    # ---- constants ----
    wih0 = singles.tile([128, G], bf16, tag="wih0")        # w_ih0.T (bf16)
    whh0 = singles.tile([128, 2, G], f32r, tag="whh0")     # w_hh0.T k-chunked
    wih1 = singles.tile([128, 2, G], f32r, tag="wih1")
    whh1 = singles.tile([128, 2, G], f32r, tag="whh1")
    wpv = singles.tile([128, 2, A + 1], f32r, tag="wpv")   # [w_p;w_v].T
    bias = singles.tile([128, 17], f32, tag="bias")
    identr = singles.tile([128, 128], f32r, tag="identr")
    identb = singles.tile([128, 128], bf16, tag="identb")
    for name, t in (("identr", identr), ("identb", identb), ("bias", bias)):
        nc.sync.dma_start(out=t, in_=io[name])
    # weights go down the ACT HWDGE ring so tile-0 input loads (sync ring)
    # are not queued behind 2.8 MB of constants
    for name, t in (("wih0", wih0), ("whh0", whh0), ("wih1", wih1),
                    ("whh1", whh1), ("wpv", wpv)):
        nc.scalar.dma_start(out=t, in_=io[name])

    def transpose(ps_out, sb_in, ident):
        kp = sb_in.partition_size()
        nc.tensor.transpose(ps_out, sb_in, ident[0:kp, 0:kp])

    def load_transpose(t):
        r0, r1 = t * nb, (t + 1) * nb
        x_bm = in_bm.tile([128, nblk, I], bf16, tag="x_bm")
        h0_bm = in_bm.tile([128, nblk, H], f32r, tag="h0_bm")
        h1_bm = in_bm.tile([128, nblk, H], f32r, tag="h1_bm")
        # SWDGE casts f32 -> bf16 during the x load
        nc.gpsimd.dma_start(out=x_bm, in_=io["x"][r0:r1, :].rearrange(
            "(b p) i -> p b i", p=128))
        nc.sync.dma_start(out=h0_bm, in_=io["h0"][r0:r1, :].rearrange(
            "(b p) j -> p b j", p=128))
        nc.sync.dma_start(out=h1_bm, in_=io["h1"][r0:r1, :].rearrange(
            "(b p) j -> p b j", p=128))

        xT = fm.tile([128, nb], bf16, tag="xT")
        h0T = fm.tile([128, 2, nb], f32r, tag="h0T")
        h1T = fm.tile([128, 2, nb], f32r, tag="h1T")
        xps_f = ps_t.tile([128, nb], f32r, tag="tp")
        xps = xps_f.bitcast(bf16)[:, 0:nb]
        for b in range(nblk):
            transpose(xps[:, b * 128:(b + 1) * 128], x_bm[:, b, :], identb)
        nc.scalar.copy(xT, xps)
        for src, dst in ((h0_bm, h0T), (h1_bm, h1T)):
            for c in range(2):
                hps = ps_t.tile([128, nb], f32r, tag="tp")
                for b in range(nblk):
                    transpose(hps[:, b * 128:(b + 1) * 128],
                              src[:, b, c * 128:(c + 1) * 128], identr)
                nc.scalar.copy(dst[:, c, :], hps)
        return {"xT": xT, "h0T": h0T, "h1T": h1T}

    def gru_layer(inp_chunks, hT, wih, whh, rz_col, in_col, hn_col,
                  houtT):
        n_in = len(inp_chunks)
        # r/z gates: 4 chunks of 128 gate rows, PSUM-accumulated
        rz = gt.tile([128, 4, nb], f32, tag="rz")
        for m in range(4):
            psg = ps_g.tile([128, nb], f32, tag="ps")
            for k in range(n_in):
                nc.tensor.matmul(psg, wih[:, k, m * 128:(m + 1) * 128]
                                 if n_in > 1 else
                                 wih[:, m * 128:(m + 1) * 128],
                                 inp_chunks[k], start=(k == 0), stop=False)
            for k in range(2):
                nc.tensor.matmul(psg, whh[:, k, m * 128:(m + 1) * 128],
                                 hT[:, k, :], start=False, stop=(k == 1))
            with tc.high_priority():
                nc.scalar.activation(rz[:, m, :], psg, Act.Sigmoid,
                                     bias=bias[:, rz_col + m:rz_col + m + 1])

        # n gate per 128-feature chunk
        n_sb = gt.tile([128, 2, nb], f32, tag="n_sb")
        d_sb = gt.tile([128, 2, nb], f32, tag="d_sb")
        p_sb = gt.tile([128, 2, nb], f32, tag="p_sb")
        for c in range(2):
            mm = 4 + c
            psi = ps_g.tile([128, nb], f32, tag="ps")
            psh = ps_g.tile([128, nb], f32, tag="ps")
            for k in range(n_in):
                nc.tensor.matmul(psi, wih[:, k, mm * 128:(mm + 1) * 128]
                                 if n_in > 1 else
                                 wih[:, mm * 128:(mm + 1) * 128],
                                 inp_chunks[k], start=(k == 0),
                                 stop=(k == n_in - 1))
            for k in range(2):
                nc.tensor.matmul(psh, whh[:, k, mm * 128:(mm + 1) * 128],
                                 hT[:, k, :], start=(k == 0), stop=(k == 1))
            # chain ops get scheduling priority so later-emitted filler
            # (next tiles' casts/copies) doesn't queue ahead of them
            with tc.high_priority():
                # p = (h_n + b_hhn) * r
                nc.vector.scalar_tensor_tensor(
                    p_sb[:, c, :], psh, bias[:, hn_col + c:hn_col + c + 1],
                    rz[:, c, :], op0=Alu.add, op1=Alu.mult)
                # u = (i_n + b_ihn) + p  (overwrites p)
                nc.vector.scalar_tensor_tensor(
                    p_sb[:, c, :], psi, bias[:, in_col + c:in_col + c + 1],
                    p_sb[:, c, :], op0=Alu.add, op1=Alu.add)
                # n = tanh(u)
                nc.scalar.activation(n_sb[:, c, :], p_sb[:, c, :], Act.Tanh)
                # h' = n + z*(h - n); the two chunks' chains run on
                # different engines (GPSIMD / DVE) concurrently
                eng = nc.gpsimd if c == 0 else nc.vector
                eng.tensor_sub(d_sb[:, c, :], asf(hT[:, c, :]),
                               n_sb[:, c, :])
                eng.tensor_mul(d_sb[:, c, :], rz[:, 2 + c, :],
                               d_sb[:, c, :])
                eng.tensor_add(houtT[:, c, :], n_sb[:, c, :],
                               d_sb[:, c, :])
        return houtT

    def layer0(t, s):
        h0nT = l1in.tile([128, 2, nb], f32r, tag="h0nT")
        gru_layer([s["xT"]], s["h0T"], wih0, whh0,
                  _RZ0, _IN0, _HN0, h0nT)
        return h0nT

    def layer1_gates(t, s, h0nT):
        h1nT = l1in.tile([128, 2, nb], f32r, tag="h1nT")
        gru_layer([h0nT[:, 0, :], h0nT[:, 1, :]], s["h1T"],
                  wih1, whh1, _RZ1, _IN1, _HN1, h1nT)
        return h1nT

    def out_store(t, hsrc, oname, li):
        r0, r1 = t * nb, (t + 1) * nb
        ho_bm = out_bm.tile([128, nblk, H], f32, tag=f"ho{li}")
        for c in range(2):
            ops = ps_t.tile([128, nb], f32r, tag="tp")
            for b in range(nblk):
                transpose(ops[:, b * 128:(b + 1) * 128],
                          hsrc[:, c, b * 128:(b + 1) * 128], identr)
            nc.vector.tensor_copy(
                ho_bm[:, :, c * 128:(c + 1) * 128],
                asf(ops).rearrange("p (b j) -> p b j", b=nblk))
        nc.scalar.dma_start(out=io[oname][r0:r1, :].rearrange(
            "(b p) j -> p b j", p=128), in_=ho_bm)

    def heads_outs(t, h0nT, h1nT):
        r0, r1 = t * nb, (t + 1) * nb
        # h0n stores first: they only need layer-0's chain, so they give
        # PE work while layer-1's chain finishes producing h1nT
        out_store(t, h0nT, "h0n", 0)

        ps3 = ps_g.tile([A + 1, nb], f32, tag="ps")
        for k in range(2):
            nc.tensor.matmul(ps3, wpv[:, k, :], h1nT[:, k, :],
                             start=(k == 0), stop=(k == 1))
        pv = gt.tile([A + 1, nb], f32, tag="pv")
        nc.scalar.activation(pv, ps3, Act.Identity,
                             bias=bias[0:A + 1, _PV:_PV + 1])
        nc.scalar.dma_start(out=io["value"][0:1, r0:r1], in_=pv[A:A + 1, :])
        nc.scalar.dma_start(out=io["logitsT"][:, r0:r1], in_=pv[0:A, :])

        out_store(t, h1nT, "h1n", 1)

    # ---- software pipeline, 2 tiles deep: between tile t's layer-1
    # matmuls and its heads/stores (which wait on the L1 gate chain), PE
    # runs tile t+2's transposes and layer-0 matmuls; tile t+1's layer-0
    # chain likewise drains behind tile t's tail work ----
    s = {0: load_transpose(0)}
    l0 = {0: layer0(0, s[0])}
    if nt > 1:
        s[1] = load_transpose(1)
        l0[1] = layer0(1, s[1])
    h1 = {}
    for t in range(nt):
        h1[t] = layer1_gates(t, s[t], l0[t])
        if t + 2 < nt:
            s[t + 2] = load_transpose(t + 2)
            l0[t + 2] = layer0(t + 2, s[t + 2])
        # heads/stores delayed one tile: by the time they enter the PE
        # stream their h1nT is long done, so PE never waits on the chain
        if t > 0:
            heads_outs(t - 1, l0[t - 1], h1[t - 1])
    heads_outs(nt - 1, l0[nt - 1], h1[nt - 1])

    ctx.close()


@functools.lru_cache(maxsize=2)
def _build(bc=BC, nb=512):
    import concourse.bacc as bacc
    import concourse.tile as tile
    from concourse import mybir

    f32 = mybir.dt.float32
    f32r = mybir.dt.float32r
    bf16 = mybir.dt.bfloat16
    nc = bacc.Bacc("TRN2", target_bir_lowering=False, debug=False)
    io = {}
    for name, shape in (("h0", [bc, H]), ("h1", [bc, H]),
                        ("whh0", [128, 2, G]),
                        ("wih1", [128, 2, G]), ("whh1", [128, 2, G]),
                        ("wpv", [128, 2, A + 1]), ("identr", [128, 128])):
        io[name] = nc.dram_tensor(name, shape, f32r,
                                  kind="ExternalInput").ap()
    io["x"] = nc.dram_tensor("x", [bc, I], f32, kind="ExternalInput").ap()
    io["wih0"] = nc.dram_tensor("wih0", [128, G], bf16,
                                kind="ExternalInput").ap()
    io["identb"] = nc.dram_tensor("identb", [128, 128], bf16,
                                  kind="ExternalInput").ap()
    io["bias"] = nc.dram_tensor("bias", [128, 17], f32,
                                kind="ExternalInput").ap()
    for name, shape in (("logitsT", [A, bc]), ("value", [1, bc]),
                        ("h0n", [bc, H]), ("h1n", [bc, H])):
        io[name] = nc.dram_tensor(name, shape, f32, kind="ExternalOutput").ap()
    with tile.TileContext(nc) as tc:
        _emit(nc, tc, io, bc, nb)
    nc.compile()
    return nc


def _prep_const(w_ih0, w_hh0, b_ih0, b_hh0, w_ih1, w_hh1, b_ih1, b_hh1,
                w_p, b_p, w_v, b_v):
    f = np.float32

    def kchunk(wT):  # [2K,M] -> [128, 2, M]
        return np.ascontiguousarray(
            wT.reshape(2, 128, wT.shape[1]).transpose(1, 0, 2))

    bias = np.zeros((128, 17), f)
    for col, vec in ((_RZ0, (b_ih0 + b_hh0)[:512]), (_RZ1, (b_ih1 + b_hh1)[:512])):
        bias[:, col:col + 4] = vec.reshape(4, 128).T
    for col, vec in ((_IN0, b_ih0[512:]), (_HN0, b_hh0[512:]),
                     (_IN1, b_ih1[512:]), (_HN1, b_hh1[512:])):
        bias[:, col:col + 2] = vec.reshape(2, 128).T
    bias[:A + 1, _PV] = np.concatenate([b_p, b_v])

    import ml_dtypes
    bf = ml_dtypes.bfloat16
    wpv = np.concatenate([w_p, w_v], axis=0).T.astype(f)  # [256, 33]
    return {
        "wih0": np.ascontiguousarray(w_ih0.T.astype(bf)),
        "whh0": kchunk(w_hh0.T.astype(f)),
        "wih1": kchunk(w_ih1.T.astype(f)),
        "whh1": kchunk(w_hh1.T.astype(f)),
        "wpv": kchunk(wpv),
        "bias": bias,
        "identr": np.eye(128, dtype=f),
        "identb": np.eye(128, dtype=bf),
    }


def kernel(x, h, w_ih0, w_hh0, b_ih0, b_hh0, w_ih1, w_hh1, b_ih1, b_hh1,
           w_p, b_p, w_v, b_v):
    global last_results
    from concourse import bass_utils

    args = [np.asarray(a, dtype=np.float32) for a in (
        x, h, w_ih0, w_hh0, b_ih0, b_hh0, w_ih1, w_hh1, b_ih1, b_hh1,
        w_p, b_p, w_v, b_v)]
    x, h = args[0], args[1]
    const = _prep_const(*args[2:])

    nc = _build()
    in_maps = []
    for c in range(NCORES):
        sl = slice(c * BC, (c + 1) * BC)
        m = dict(const)
        m["x"] = np.ascontiguousarray(x[sl, -1, :])
        m["h0"] = np.ascontiguousarray(h[0, sl, :])
        m["h1"] = np.ascontiguousarray(h[1, sl, :])
        in_maps.append(m)

    trace = bool(int(os.environ.get("KERNEL_PROFILE", "0")))
    res = bass_utils.run_bass_kernel_spmd(
        nc, in_maps, core_ids=list(range(NCORES)), trace=trace)
    last_results = res

    logits = np.concatenate(
        [np.ascontiguousarray(r["logitsT"].T) for r in res.results], axis=0)
    value = np.concatenate([r["value"][0] for r in res.results], axis=0)
    h_new = np.stack([
        np.concatenate([r["h0n"] for r in res.results], axis=0),
        np.concatenate([r["h1n"] for r in res.results], axis=0)], axis=0)
    return logits, value, h_new


# revision 43
# speedup vs baseline: 1.2177x; 1.0025x over previous
"""Trainium2 Bass kernel for a 2-layer GRU cell (seq len 1) + actor/critic heads.

Reference computation (PyTorch GRU gate order r,z,n), B=32768, I=128, H=256:
    h0' = GRUCell(x,  h[0]; w_ih0, w_hh0, b_ih0, b_hh0)
    h1' = GRUCell(h0', h[1]; w_ih1, w_hh1, b_ih1, b_hh1)
    logits = h1' @ w_p.T + b_p ; value = h1' @ w_v.T + b_v
Outputs: (logits [B,32], value [B], h_new [2,B,256])

Strategy: pure data parallel over 8 NeuronCores (4096 rows each).  On-chip
compute is feature-major (features on partitions, batch on the free dim):
batch-major DRAM tiles are transposed on the PE, all matmuls run in
float32r (full-rate on TRN2; fp32 matmul is 4 cycles/row), gate math is
fp32 on ACT/DVE/GPSIMD, outputs are PE-transposed back to batch-major.
"""

import functools
import os
import sys

import numpy as np

for _p in ("/opt/trn_rl_repo", "/root/.axon_site/_ro/trn_rl_repo"):
    if os.path.isdir(_p) and _p not in sys.path:
        sys.path.insert(0, _p)

B, S, I, H, A, L = 32768, 1, 128, 256, 32, 2
G = 3 * H  # 768 gate rows
NCORES = 8
BC = B // NCORES  # batch rows per core

# bias pack column indices
_RZ0, _IN0, _HN0 = 0, 4, 6
_RZ1, _IN1, _HN1 = 8, 12, 14
_PV = 16

last_results = None  # BassKernelResults of the most recent run (for profiling)


def _emit(nc, tc, io, bc, nb):
    """Emit the per-core program as a 3-stage software pipeline over tiles:
    S1 load+transpose(t+1) | S2 layer0(t+1) | S3 layer1+heads+stores(t),
    so PE crunches tile t+1 transposes/matmuls while tile t's gate math
    drains through ACT/DVE/GPSIMD."""
    import concourse.bass as bass  # noqa: F401
    from contextlib import ExitStack
    from concourse import mybir

    ctx = ExitStack()

    f32 = mybir.dt.float32
    f32r = mybir.dt.float32r
    bf16 = mybir.dt.bfloat16
    Act = mybir.ActivationFunctionType
    Alu = mybir.AluOpType
    nt = bc // nb
    nblk = nb // 128  # 128-row blocks per batch tile

    def asf(ap):  # read an fp32r tile as plain fp32 (free bit view)
        return ap.bitcast(f32)

    singles = ctx.enter_context(tc.tile_pool(name="singles", bufs=1))
    in_bm = ctx.enter_context(tc.tile_pool(name="in_bm", bufs=3))
    fm = ctx.enter_context(tc.tile_pool(name="fm", bufs=3))
    l1in = ctx.enter_context(tc.tile_pool(name="l1in", bufs=3))
    gt = ctx.enter_context(tc.tile_pool(name="gt", bufs=2))
    out_bm = ctx.enter_context(tc.tile_pool(name="out_bm", bufs=2))
    ps_t = ctx.enter_context(tc.tile_pool(name="ps_t", bufs=4, space="PSUM"))
    ps_g = ctx.enter_context(tc.tile_pool(name="ps_g", bufs=4, space="PSUM"))

    # ---- constants ----
    wih0 = singles.tile([128, G], bf16, tag="wih0")        # w_ih0.T (bf16)
    whh0 = singles.tile([128, 2, G], f32r, tag="whh0")     # w_hh0.T k-chunked
    wih1 = singles.tile([128, 2, G], f32r, tag="wih1")
    whh1 = singles.tile([128, 2, G], f32r, tag="whh1")
    wpv = singles.tile([128, 2, A + 1], f32r, tag="wpv")   # [w_p;w_v].T
    bias = singles.tile([128, 17], f32, tag="bias")
    identr = singles.tile([128, 128], f32r, tag="identr")
    identb = singles.tile([128, 128], bf16, tag="identb")
    for name, t in (("identr", identr), ("identb", identb), ("bias", bias)):
        nc.sync.dma_start(out=t, in_=io[name])
    # weights go down the ACT HWDGE ring so tile-0 input loads (sync ring)
    # are not queued behind 2.8 MB of constants
    for name, t in (("wih0", wih0), ("whh0", whh0), ("wih1", wih1),
                    ("whh1", whh1), ("wpv", wpv)):
        nc.scalar.dma_start(out=t, in_=io[name])

    def transpose(ps_out, sb_in, ident):
        kp = sb_in.partition_size()
        nc.tensor.transpose(ps_out, sb_in, ident[0:kp, 0:kp])

    def load_transpose(t):
        r0, r1 = t * nb, (t + 1) * nb
        x_bm = in_bm.tile([128, nblk, I], bf16, tag="x_bm")
        h0_bm = in_bm.tile([128, nblk, H], f32r, tag="h0_bm")
        h1_bm = in_bm.tile([128, nblk, H], f32r, tag="h1_bm")
        # SWDGE casts f32 -> bf16 during the x load
        nc.gpsimd.dma_start(out=x_bm, in_=io["x"][r0:r1, :].rearrange(
            "(b p) i -> p b i", p=128))
        nc.sync.dma_start(out=h0_bm, in_=io["h0"][r0:r1, :].rearrange(
            "(b p) j -> p b j", p=128))
        nc.sync.dma_start(out=h1_bm, in_=io["h1"][r0:r1, :].rearrange(
            "(b p) j -> p b j", p=128))

        xT = fm.tile([128, nb], bf16, tag="xT")
        h0T = fm.tile([128, 2, nb], f32r, tag="h0T")
        h1T = fm.tile([128, 2, nb], f32r, tag="h1T")
        xps_f = ps_t.tile([128, nb], f32r, tag="tp")
        xps = xps_f.bitcast(bf16)[:, 0:nb]
        for b in range(nblk):
            transpose(xps[:, b * 128:(b + 1) * 128], x_bm[:, b, :], identb)
        nc.scalar.copy(xT, xps)
        for src, dst in ((h0_bm, h0T), (h1_bm, h1T)):
            for c in range(2):
                hps = ps_t.tile([128, nb], f32r, tag="tp")
                for b in range(nblk):
                    transpose(hps[:, b * 128:(b + 1) * 128],
                              src[:, b, c * 128:(c + 1) * 128], identr)
                nc.scalar.copy(dst[:, c, :], hps)
        return {"xT": xT, "h0T": h0T, "h1T": h1T}

    def gru_layer(inp_chunks, hT, wih, whh, rz_col, in_col, hn_col,
                  houtT):
        n_in = len(inp_chunks)
        # r/z gates: 4 chunks of 128 gate rows, PSUM-accumulated
        rz = gt.tile([128, 4, nb], f32, tag="rz")
        for m in range(4):
            psg = ps_g.tile([128, nb], f32, tag="ps")
            for k in range(n_in):
                nc.tensor.matmul(psg, wih[:, k, m * 128:(m + 1) * 128]
                                 if n_in > 1 else
                                 wih[:, m * 128:(m + 1) * 128],
                                 inp_chunks[k], start=(k == 0), stop=False)
            for k in range(2):
                nc.tensor.matmul(psg, whh[:, k, m * 128:(m + 1) * 128],
                                 hT[:, k, :], start=False, stop=(k == 1))
            with tc.high_priority():
                nc.scalar.activation(rz[:, m, :], psg, Act.Sigmoid,
                                     bias=bias[:, rz_col + m:rz_col + m + 1])

        # n gate per 128-feature chunk
        n_sb = gt.tile([128, 2, nb], f32, tag="n_sb")
        d_sb = gt.tile([128, 2, nb], f32, tag="d_sb")
        p_sb = gt.tile([128, 2, nb], f32, tag="p_sb")
        for c in range(2):
            mm = 4 + c
            psi = ps_g.tile([128, nb], f32, tag="ps")
            psh = ps_g.tile([128, nb], f32, tag="ps")
            for k in range(n_in):
                nc.tensor.matmul(psi, wih[:, k, mm * 128:(mm + 1) * 128]
                                 if n_in > 1 else
                                 wih[:, mm * 128:(mm + 1) * 128],
                                 inp_chunks[k], start=(k == 0),
                                 stop=(k == n_in - 1))
            for k in range(2):
                nc.tensor.matmul(psh, whh[:, k, mm * 128:(mm + 1) * 128],
                                 hT[:, k, :], start=(k == 0), stop=(k == 1))
            # chain ops get scheduling priority so later-emitted filler
            # (next tiles' casts/copies) doesn't queue ahead of them
            with tc.high_priority():
                # p = (h_n + b_hhn) * r
                nc.vector.scalar_tensor_tensor(
                    p_sb[:, c, :], psh, bias[:, hn_col + c:hn_col + c + 1],
                    rz[:, c, :], op0=Alu.add, op1=Alu.mult)
                # u = (i_n + b_ihn) + p  (overwrites p)
                nc.vector.scalar_tensor_tensor(
                    p_sb[:, c, :], psi, bias[:, in_col + c:in_col + c + 1],
                    p_sb[:, c, :], op0=Alu.add, op1=Alu.add)
                # n = tanh(u)
                nc.scalar.activation(n_sb[:, c, :], p_sb[:, c, :], Act.Tanh)
                # h' = n + z*(h - n); the two chunks' chains run on
                # different engines (GPSIMD / DVE) concurrently
                eng = nc.gpsimd if c == 0 else nc.vector
                eng.tensor_sub(d_sb[:, c, :], asf(hT[:, c, :]),
                               n_sb[:, c, :])
                eng.tensor_mul(d_sb[:, c, :], rz[:, 2 + c, :],
                               d_sb[:, c, :])
                eng.tensor_add(houtT[:, c, :], n_sb[:, c, :],
                               d_sb[:, c, :])
        return houtT

    def layer0(t, s):
        h0nT = l1in.tile([128, 2, nb], f32r, tag="h0nT")
        gru_layer([s["xT"]], s["h0T"], wih0, whh0,
                  _RZ0, _IN0, _HN0, h0nT)
        return h0nT

    def layer1_gates(t, s, h0nT):
        h1nT = l1in.tile([128, 2, nb], f32r, tag="h1nT")
        gru_layer([h0nT[:, 0, :], h0nT[:, 1, :]], s["h1T"],
                  wih1, whh1, _RZ1, _IN1, _HN1, h1nT)
        return h1nT

    def out_store(t, hsrc, oname, li):
        r0, r1 = t * nb, (t + 1) * nb
        ho_bm = out_bm.tile([128, nblk, H], f32, tag=f"ho{li}")
        for c in range(2):
            ops = ps_t.tile([128, nb], f32r, tag="tp")
            for b in range(nblk):
                transpose(ops[:, b * 128:(b + 1) * 128],
                          hsrc[:, c, b * 128:(b + 1) * 128], identr)
            nc.vector.tensor_copy(
                ho_bm[:, :, c * 128:(c + 1) * 128],
                asf(ops).rearrange("p (b j) -> p b j", b=nblk))
        nc.scalar.dma_start(out=io[oname][r0:r1, :].rearrange(
            "(b p) j -> p b j", p=128), in_=ho_bm)

    def heads_outs(t, h0nT, h1nT):
        r0, r1 = t * nb, (t + 1) * nb
        # h0n stores first: they only need layer-0's chain, so they give
        # PE work while layer-1's chain finishes producing h1nT
        out_store(t, h0nT, "h0n", 0)

        ps3 = ps_g.tile([A + 1, nb], f32, tag="ps")
        for k in range(2):
            nc.tensor.matmul(ps3, wpv[:, k, :], h1nT[:, k, :],
                             start=(k == 0), stop=(k == 1))
        pv = gt.tile([A + 1, nb], f32, tag="pv")
        nc.scalar.activation(pv, ps3, Act.Identity,
                             bias=bias[0:A + 1, _PV:_PV + 1])
        nc.scalar.dma_start(out=io["value"][0:1, r0:r1], in_=pv[A:A + 1, :])
        nc.scalar.dma_start(out=io["logitsT"][:, r0:r1], in_=pv[0:A, :])

        out_store(t, h1nT, "h1n", 1)

    # ---- software pipeline, 2 tiles deep: between tile t's layer-1
    # matmuls and its heads/stores (which wait on the L1 gate chain), PE
    # runs tile t+2's transposes and layer-0 matmuls; tile t+1's layer-0
    # chain likewise drains behind tile t's tail work ----
    s = {0: load_transpose(0)}
    l0 = {0: layer0(0, s[0])}
    if nt > 1:
        s[1] = load_transpose(1)
        l0[1] = layer0(1, s[1])
    h1 = {}
    for t in range(nt):
        h1[t] = layer1_gates(t, s[t], l0[t])
        if t + 2 < nt:
            s[t + 2] = load_transpose(t + 2)
            l0[t + 2] = layer0(t + 2, s[t + 2])
        # heads/stores delayed one tile: by the time they enter the PE
        # stream their h1nT is long done, so PE never waits on the chain
        if t > 0:
            heads_outs(t - 1, l0[t - 1], h1[t - 1])
    heads_outs(nt - 1, l0[nt - 1], h1[nt - 1])

    ctx.close()


@functools.lru_cache(maxsize=2)
def _build(bc=BC, nb=512):
    import concourse.bacc as bacc
    import concourse.tile as tile
    from concourse import mybir

    f32 = mybir.dt.float32
    f32r = mybir.dt.float32r
    bf16 = mybir.dt.bfloat16
    nc = bacc.Bacc("TRN2", target_bir_lowering=False, debug=False)
    io = {}
    for name, shape in (("h0", [bc, H]), ("h1", [bc, H]),
                        ("whh0", [128, 2, G]),
                        ("wih1", [128, 2, G]), ("whh1", [128, 2, G]),
                        ("wpv", [128, 2, A + 1]), ("identr", [128, 128])):
        io[name] = nc.dram_tensor(name, shape, f32r,
                                  kind="ExternalInput").ap()
    io["x"] = nc.dram_tensor("x", [bc, I], f32, kind="ExternalInput").ap()
    io["wih0"] = nc.dram_tensor("wih0", [128, G], bf16,
                                kind="ExternalInput").ap()
    io["identb"] = nc.dram_tensor("identb", [128, 128], bf16,
                                  kind="ExternalInput").ap()
    io["bias"] = nc.dram_tensor("bias", [128, 17], f32,
                                kind="ExternalInput").ap()
    for name, shape in (("logitsT", [A, bc]), ("value", [1, bc]),
                        ("h0n", [bc, H]), ("h1n", [bc, H])):
        io[name] = nc.dram_tensor(name, shape, f32, kind="ExternalOutput").ap()
    with tile.TileContext(nc) as tc:
        _emit(nc, tc, io, bc, nb)
    nc.compile()
    return nc


def _prep_const(w_ih0, w_hh0, b_ih0, b_hh0, w_ih1, w_hh1, b_ih1, b_hh1,
                w_p, b_p, w_v, b_v):
    f = np.float32

    def kchunk(wT):  # [2K,M] -> [128, 2, M]
        return np.ascontiguousarray(
            wT.reshape(2, 128, wT.shape[1]).transpose(1, 0, 2))

    bias = np.zeros((128, 17), f)
    for col, vec in ((_RZ0, (b_ih0 + b_hh0)[:512]), (_RZ1, (b_ih1 + b_hh1)[:512])):
        bias[:, col:col + 4] = vec.reshape(4, 128).T
    for col, vec in ((_IN0, b_ih0[512:]), (_HN0, b_hh0[512:]),
                     (_IN1, b_ih1[512:]), (_HN1, b_hh1[512:])):
        bias[:, col:col + 2] = vec.reshape(2, 128).T
    bias[:A + 1, _PV] = np.concatenate([b_p, b_v])

    import ml_dtypes
    bf = ml_dtypes.bfloat16
    wpv = np.concatenate([w_p, w_v], axis=0).T.astype(f)  # [256, 33]
    return {
        "wih0": np.ascontiguousarray(w_ih0.T.astype(bf)),
        "whh0": kchunk(w_hh0.T.astype(f)),
        "wih1": kchunk(w_ih1.T.astype(f)),
        "whh1": kchunk(w_hh1.T.astype(f)),
        "wpv": kchunk(wpv),
        "bias": bias,
        "identr": np.eye(128, dtype=f),
        "identb": np.eye(128, dtype=bf),
    }


def kernel(x, h, w_ih0, w_hh0, b_ih0, b_hh0, w_ih1, w_hh1, b_ih1, b_hh1,
           w_p, b_p, w_v, b_v):
    global last_results
    from concourse import bass_utils

    args = [np.asarray(a, dtype=np.float32) for a in (
        x, h, w_ih0, w_hh0, b_ih0, b_hh0, w_ih1, w_hh1, b_ih1, b_hh1,
        w_p, b_p, w_v, b_v)]
    x, h = args[0], args[1]
    const = _prep_const(*args[2:])

    nc = _build()
    in_maps = []
    for c in range(NCORES):
        sl = slice(c * BC, (c + 1) * BC)
        m = dict(const)
        m["x"] = np.ascontiguousarray(x[sl, -1, :])
        m["h0"] = np.ascontiguousarray(h[0, sl, :])
        m["h1"] = np.ascontiguousarray(h[1, sl, :])
        in_maps.append(m)

    trace = bool(int(os.environ.get("KERNEL_PROFILE", "0")))
    res = bass_utils.run_bass_kernel_spmd(
        nc, in_maps, core_ids=list(range(NCORES)), trace=trace)
    last_results = res

    logits = np.concatenate(
        [np.ascontiguousarray(r["logitsT"].T) for r in res.results], axis=0)
    value = np.concatenate([r["value"][0] for r in res.results], axis=0)
    h_new = np.stack([
        np.concatenate([r["h0n"] for r in res.results], axis=0),
        np.concatenate([r["h1n"] for r in res.results], axis=0)], axis=0)
    return logits, value, h_new
